# revision 15
# baseline (speedup 1.0000x reference)
"""Causal self-attention (B=4, T=2048, C=1024, H=16) on 8 TRN2 NeuronCores.

Sharding: tensor-parallel over heads — 2 heads per core. Each core:
  - computes Q^T,K^T (head-dim on partitions) and V (token-dim on partitions)
    for its 2 heads from the full input x,
  - runs causal attention in transposed-score layout S^T[k, q] so the softmax
    denominator comes for free from a ones-column appended to V,
  - computes a partial output  y_local @ w_proj[:, c_slice]^T  over its 128
    channels.
Host sums the 8 partials (the all-reduce of the row-sharded projection).

Matmuls run in bf16 (fp32 PSUM accumulation); softmax runs in fp32 on the
scalar engine.  exp() is computed without max-subtraction: scores for randn
inputs are O(4) after the 1/8 scale, far below fp32 overflow.
"""

import numpy as np
import ml_dtypes

B, T, C, H = 4, 2048, 1024, 16
HD = C // H            # 64 head dim
NCORES = 8
HPC = H // NCORES      # 2 heads per core
RPC = HPC * HD         # 128 rows (channels) per core for each of q/k/v
BT = B * T             # 8192
CT = C // 128          # 8 contraction tiles
QCH = 1024             # q-chunk width (ACT/psum granularity)
NCH = T // QCH         # 2 chunks per (b, h)
KPC = QCH // 128       # 8 k-tiles per chunk
NTT = T // 128         # 16 token tiles per batch

_prog_cache = {}


def build_program():
    """Build the (SPMD-identical) Bass program. Inputs differ per core."""
    from contextlib import ExitStack
    import concourse.bass as bass
    import concourse.mybir as mybir
    import concourse.tile as tile
    from concourse import bacc

    f32 = mybir.dt.float32
    bf16 = mybir.dt.bfloat16
    EXP = mybir.ActivationFunctionType.Exp

    nc = bacc.Bacc("TRN2", target_bir_lowering=False, debug=False)

    xt = nc.dram_tensor("xt", [CT, B, 128, T], bf16, kind="ExternalInput").ap()
    wqkv = nc.dram_tensor("wqkv", [CT, 128, 3 * RPC], bf16, kind="ExternalInput").ap()
    wproj = nc.dram_tensor("wproj", [128, C], bf16, kind="ExternalInput").ap()
    outp = nc.dram_tensor("outp", [BT, C], bf16, kind="ExternalOutput").ap()

    with tile.TileContext(nc) as tc, ExitStack() as ctx:
        const = ctx.enter_context(tc.tile_pool(name="const", bufs=1))
        qk_pool = ctx.enter_context(tc.tile_pool(name="qkp", bufs=2))
        v_pool = ctx.enter_context(tc.tile_pool(name="vp", bufs=2))
        pt_pool = ctx.enter_context(tc.tile_pool(name="ptp", bufs=2))
        sm_pool = ctx.enter_context(tc.tile_pool(name="smp", bufs=1))
        st_pool = ctx.enter_context(tc.tile_pool(name="stp", bufs=2))
        ps = ctx.enter_context(tc.tile_pool(name="ps", bufs=2, space="PSUM"))
        ps_y = ctx.enter_context(tc.tile_pool(name="psy", bufs=1, space="PSUM"))

        # ---- constants ----
        wqkv_sb = const.tile([128, CT, 3 * RPC], bf16, tag="wqkv")
        nc.sync.dma_start(out=wqkv_sb, in_=wqkv.rearrange("ct p r -> p ct r"))
        wproj_sb = const.tile([128, C], bf16, tag="wproj")
        nc.sync.dma_start(out=wproj_sb, in_=wproj)

        ident = const.tile([128, 128], bf16, tag="ident")
        from concourse.masks import make_identity
        make_identity(nc, ident)

        ones_sb = const.tile([128, HD], f32, tag="ones")
        nc.vector.memset(ones_sb, 1.0)

        # stage all of x^T in SBUF once (64KB/partition) — each region is
        # written exactly once so no DMA ever carries a WAR/WAW wait.
        xt_sb = const.tile([128, B, CT, T], bf16, tag="xts")
        for b in range(B):
            for c in range(CT):
                nc.sync.dma_start(out=xt_sb[:, b, c, :], in_=xt[c, b])

        # tri[p, f] = 1.0 where p <= f else 0 (keep k <= q in transposed scores)
        tri = const.tile([128, 128], bf16, tag="tri")
        nc.gpsimd.memset(tri, 1.0)
        nc.gpsimd.affine_select(
            out=tri, in_=tri,
            compare_op=mybir.AluOpType.is_ge,
            fill=0.0, base=0,
            channel_multiplier=-1,       # expr = -p + f >= 0  -> keep
            pattern=[[1, 128]],
        )

        for b in range(B):
            # ---------- QKV projection for batch b ----------
            qt_b = qk_pool.tile([128, T], bf16, tag="qt", name=f"qt_{b}")
            kt_b = qk_pool.tile([128, T], bf16, tag="kt", name=f"kt_{b}")
            vt_b = qk_pool.tile([128, T], bf16, tag="vt", name=f"vt_{b}")
            yl_b = qk_pool.tile([128, T], bf16, tag="yl", name=f"yl_{b}")
            dests = [qt_b, kt_b, vt_b]
            for rg in range(3):
                for q5 in range(T // 512):
                    acc = ps.tile([128, 512], f32, tag="m", name=f"qkv_{b}_{rg}_{q5}")
                    for c in range(CT):
                        nc.tensor.matmul(
                            acc,
                            lhsT=wqkv_sb[:, c, rg * 128:(rg + 1) * 128],
                            rhs=xt_sb[:, b, c, q5 * 512:(q5 + 1) * 512],
                            start=(c == 0), stop=(c == CT - 1),
                        )
                    nc.vector.tensor_copy(dests[rg][:, q5 * 512:(q5 + 1) * 512], acc)

            # ---------- V in [token, dim] layout with ones column ----------
            v_b = v_pool.tile([128, NTT, HPC, HD + 1], bf16, tag="v", name=f"v_{b}")
            nc.vector.memset(v_b[:, :, :, HD:HD + 1], 1.0)
            for tt in range(NTT):
                vtr = ps.tile([128, 128], bf16, tag="m", name=f"vtr_{b}_{tt}")
                nc.tensor.transpose(vtr, vt_b[:, tt * 128:(tt + 1) * 128], ident)
                for h in range(HPC):
                    nc.vector.tensor_copy(
                        v_b[:, tt, h, 0:HD], vtr[:, h * HD:(h + 1) * HD])

            # ---------- attention per head ----------
            for h in range(HPC):
                hp = h * HD                    # partition base of this head in qt/kt
                for ch in range(NCH):
                    q0 = ch * QCH
                    nkt = KPC * (ch + 1)       # k tiles 0..nkt-1
                    yaug = ps_y.tile([HD + 1, QCH], f32, tag=f"y{ch % 2}",
                                     name=f"yaug_{b}_{h}_{ch}")
                    for j in range(nkt):
                        m = j - KPC * ch       # diag block index (>=0 on diagonal band)
                        lo = max(0, m) * 128   # first needed q col in chunk
                        s_ps = ps.tile([128, QCH], f32, tag="m",
                                       name=f"s_{b}_{h}_{ch}_{j}")
                        for s0, s1 in ((lo, 512), (512, QCH)):
                            a, e = max(lo, s0), min(QCH, s1)
                            if a >= e:
                                continue
                            nc.tensor.matmul(
                                s_ps[:, a:e],
                                lhsT=kt_b[hp:hp + HD, j * 128:(j + 1) * 128],
                                rhs=qt_b[hp:hp + HD, q0 + a:q0 + e],
                                start=True, stop=True,
                            )
                        p_t = pt_pool.tile([128, QCH], bf16, tag="pt",
                                           name=f"pt_{b}_{h}_{ch}_{j}")
                        nc.scalar.activation(
                            p_t[:, lo:QCH], s_ps[:, lo:QCH], EXP, scale=1.0 / 8.0)
                        if m >= 0:
                            nc.vector.tensor_mul(
                                p_t[:, lo:lo + 128], p_t[:, lo:lo + 128], tri)
                        # PV: accumulate Y_aug[d+1, q] over k tiles
                        last1 = min(nkt - 1, KPC * ch + 3)   # last j touching cols <512
                        for s0, s1, last in ((lo, 512, last1), (512, QCH, nkt - 1)):
                            a, e = max(lo, s0), min(QCH, s1)
                            if a >= e:
                                continue
                            nc.tensor.matmul(
                                yaug[:, a:e],
                                lhsT=v_b[:, j, h, :],
                                rhs=p_t[:, a:e],
                                start=(j == 0), stop=(j == last),
                            )
                    # normalize: y = y / l ; l = yaug[HD]
                    r_sb = sm_pool.tile([128, QCH], f32, tag="r",
                                        name=f"r_{b}_{h}_{ch}")
                    nc.vector.reciprocal(r_sb[HD:HD + 1, :], yaug[HD:HD + 1, :])
                    # broadcast r across 64 partitions: PE outer product ones^T r
                    rps = ps.tile([HD, QCH], f32, tag="m", name=f"rps_{b}_{h}_{ch}")
                    for s0 in range(0, QCH, 512):
                        nc.tensor.matmul(
                            rps[:, s0:s0 + 512],
                            lhsT=ones_sb[HD:HD + 1, :],
                            rhs=r_sb[HD:HD + 1, s0:s0 + 512],
                            start=True, stop=True,
                        )
                    rb_sb = sm_pool.tile([HD, QCH], f32, tag="rb",
                                         name=f"rb_{b}_{h}_{ch}")
                    nc.vector.tensor_copy(rb_sb, rps)
                    if h == 0:
                        ydst = yl_b[0:HD, q0:q0 + QCH]
                    else:
                        ytmp = sm_pool.tile([HD, QCH], bf16, tag="ytmp",
                                            name=f"ytmp_{b}_{ch}")
                        ydst = ytmp
                    nc.vector.tensor_mul(ydst, yaug[0:HD, :], rb_sb)
                    if h == 1:
                        nc.gpsimd.dma_start(
                            out=yl_b[HD:2 * HD, q0:q0 + QCH], in_=ytmp)

            # ---------- output projection (partial over this core's channels) ----
            for tt in range(NTT):
                op = ps.tile([128, C], f32, tag="m", name=f"op_{b}_{tt}")
                for n5 in range(C // 512):
                    nc.tensor.matmul(
                        op[:, n5 * 512:(n5 + 1) * 512],
                        lhsT=yl_b[:, tt * 128:(tt + 1) * 128],
                        rhs=wproj_sb[:, n5 * 512:(n5 + 1) * 512],
                        start=True, stop=True,
                    )
                o_sb = st_pool.tile([128, C], bf16, tag="o", name=f"o_{b}_{tt}")
                nc.vector.tensor_copy(o_sb, op)
                nc.gpsimd.dma_start(
                    out=outp[b * T + tt * 128: b * T + (tt + 1) * 128, :], in_=o_sb)

    nc.compile()
    return nc


def _prep_inputs(x, w_attn, w_proj):
    """Host-side sharding: build per-core input maps."""
    bf16 = ml_dtypes.bfloat16
    x = np.asarray(x, dtype=np.float32)
    w_attn = np.asarray(w_attn, dtype=np.float32)
    w_proj = np.asarray(w_proj, dtype=np.float32)

    # x^T tiles: [CT, B, 128, T]
    xt = np.ascontiguousarray(
        x.reshape(BT, C).T.reshape(CT, 128, B, T).transpose(0, 2, 1, 3)
    ).astype(bf16)

    in_maps = []
    for g in range(NCORES):
        r0 = g * RPC
        w_local = np.concatenate([
            w_attn[r0:r0 + RPC],              # q rows of heads 2g, 2g+1
            w_attn[C + r0:C + r0 + RPC],      # k rows
            w_attn[2 * C + r0:2 * C + r0 + RPC],  # v rows
        ], axis=0)                            # [384, C]
        wqkv = np.ascontiguousarray(
            w_local.T.reshape(CT, 128, 3 * RPC)).astype(bf16)
        wprojT = np.ascontiguousarray(w_proj[:, r0:r0 + RPC].T).astype(bf16)
        in_maps.append({"xt": xt, "wqkv": wqkv, "wproj": wprojT})
    return in_maps


def kernel(x, w_attn, w_proj):
    from concourse import bass_utils

    if "nc" not in _prog_cache:
        _prog_cache["nc"] = build_program()
    nc = _prog_cache["nc"]

    in_maps = _prep_inputs(x, w_attn, w_proj)
    res = bass_utils.run_bass_kernel_spmd(
        nc, in_maps, core_ids=list(range(NCORES)))

    acc = np.zeros((BT, C), dtype=np.float32)
    for g in range(NCORES):
        part = np.asarray(res.results[g]["outp"])
        if part.dtype != np.float32:
            # bf16 -> f32 exact upcast via bit manipulation (fast on host)
            part = (part.view(np.uint16).astype(np.uint32) << 16).view(np.float32)
        acc += part
    return acc.reshape(B, T, C)
